# revision 11
# baseline (speedup 1.0000x reference)
"""Masked ternary linear layer on 8 TRN2 NeuronCores.

out = x @ ternarize((weight_base + weight_tag) * expand(tile_mask)).T + bias

Sharding: tensor-parallel column sharding along out_features. Each core
gets a 1024-wide slice of the weights, x is replicated; the 8 per-core
[128, 1024] outputs are concatenated on host.

The ternary weight matrix (values in {-1,0,+1}) is a pure function of
the inputs, computed once on the host exactly as the reference does and
shipped to the device packed 2 bits per weight. The 2-bit code of each
weight is chosen so that, shifted to the top bits of its byte, it IS
the fp8e4m3 bit pattern of 2*w:

   w=+1 -> 01 -> 0x40 = +2.0    w=-1 -> 11 -> 0xC0 = -2.0    w=0 -> 00

Four weights (4 k-chunks) pack per byte; uint16 lanes cover two
adjacent output columns. Device decode is ONE fused DVE op per
[128, 1024] weight plane, on uint16 lanes at half the element count:

   plane_q = (packed << 2q) & 0xC0C0        # uint16 in/out, bitVec ops

and the result is bitcast to fp8e4 for the matmul (moving operand fp8,
full PE rate). x is pre-scaled by 0.5 on the host so x/2 @ (2w) = x@w;
bias is seeded into PSUM with a K=1 ones matmul before the loop.

Per-core budget: DMA 2.1 MB packed weights + 2.1 MB x + 0.5 MB out
~ 14 us; DVE 64 decode ops ~ 13-21 us; PE 64x1024 cols ~ 27.5 us.
PE-bound by design.
"""

import numpy as np

import concourse.bass as bass
import concourse.mybir as mybir
from concourse import bacc
from concourse.bass_utils import run_bass_kernel_spmd
from concourse.tile import TileContext

N_CORES = 8
BATCH = 128
IN_FEATURES = 8192
OUT_FEATURES = 8192
TILE = 64
THRESH = 0.3
OUT_CORE = OUT_FEATURES // N_CORES

_F32 = mybir.dt.float32
_BF16 = mybir.dt.bfloat16
_U16 = mybir.dt.uint16
_FP8 = mybir.dt.float8e4


def _ternarize(weight_base, weight_tag, tile_mask):
    """Exact reference ternarization (f32 ops), -> int8 in {-1,0,1}."""
    mask = np.repeat(np.repeat(tile_mask, TILE, 0), TILE, 1).astype(np.float32)
    w = np.clip((weight_base + weight_tag) * mask, -1.0, 1.0)
    w = np.where(np.abs(w) < THRESH, 0.0, np.sign(w))
    return w.astype(np.int8)


def build_graph(in_features: int, out_core: int, batch: int = BATCH,
                mode: str = "p2f8",       # p2f8 | fp8
                wd_bufs: int = 12, pk_bufs: int = 3) -> bacc.Bacc:
    KC = in_features // 128          # contraction chunks of 128 rows
    GC = KC // 4                     # packed groups (4 chunks per byte)
    LAN = out_core // 2              # uint16 lanes per packed plane

    nc = bacc.Bacc("TRN2", target_bir_lowering=False, debug=False,
                   num_devices=N_CORES)
    # xTc[p, k*batch + b] = x[b, k*128 + p] * 0.5   (bf16)
    xTc = nc.dram_tensor("xTc", [128, KC * batch], _BF16,
                         kind="ExternalInput").ap()
    if mode == "p2f8":
        wpk = nc.dram_tensor("wpk", [128, GC * LAN], _U16,
                             kind="ExternalInput").ap()
    else:
        w8 = nc.dram_tensor("w8", [128, KC * out_core], _FP8,
                            kind="ExternalInput").ap()
    bias = nc.dram_tensor("bias", [1, out_core], _BF16,
                          kind="ExternalInput").ap()
    out = nc.dram_tensor("out", [batch, out_core], _BF16,
                         kind="ExternalOutput").ap()

    slices = [(o, min(512, out_core - o)) for o in range(0, out_core, 512)]
    # x DMA pieces (in chunks): fine-grained at the front so the PE can
    # start as soon as piece 0 lands
    xpieces = [4, 4, 8, 16, 16, 16]
    assert sum(xpieces) == KC

    with TileContext(nc) as tc:
        with (
            tc.tile_pool(name="persist", bufs=1) as persist,
            tc.tile_pool(name="pk", bufs=pk_bufs) as pkp,
            tc.tile_pool(name="wd", bufs=wd_bufs) as wdp,
            tc.tile_pool(name="outp", bufs=1) as outp,
            tc.tile_pool(name="psum", bufs=1, space="PSUM") as psp,
        ):
            # Deadline-ordered DMA schedule on the two HWDGE rings.
            # ring A (sync):   pk0, bias, pk2, pk4, ...
            # ring B (scalar): x0, pk1, x1, pk3, x2, pk5, x3, pk7, x4, ...
            bias_sb = persist.tile([1, out_core], _BF16)
            xT_sb = persist.tile([128, KC, batch], _BF16)
            pk_ts = []

            def dma_pk(g):
                pk_t = pkp.tile([128, LAN], _U16, name=f"pk{g}")
                q = nc.sync if len(pk_ts) % 2 == 0 else nc.scalar
                q.dma_start(out=pk_t[:],
                            in_=wpk[:, g * LAN:(g + 1) * LAN])
                pk_ts.append(pk_t)

            def dma_x(p, q):
                a = sum(xpieces[:p])
                b = a + xpieces[p]
                q.dma_start(
                    out=xT_sb[:, a:b, :],
                    in_=xTc[:, a * batch:b * batch].rearrange(
                        "p (k b) -> p k b", b=batch))

            if mode == "p2f8":
                # ring A (sync):   pk0 x2 pk2 pk4 x4 pk6 pk8 x5 pk10 ...
                # ring B (scalar): bias x0 pk1 x1 pk3 pk5 x3 pk7 pk9 ...
                nc.scalar.dma_start(out=bias_sb[:], in_=bias[:])  # B tiny
                dma_pk(0)                      # A
                dma_x(0, nc.scalar)            # B
                dma_pk(1)                      # B
                dma_x(1, nc.scalar)            # B
                dma_x(2, nc.sync)              # A
                dma_pk(2)                      # A
                dma_pk(3)                      # B
                dma_pk(4)                      # A
                dma_pk(5)                      # B
                dma_x(4, nc.sync)              # A
                dma_x(3, nc.scalar)            # B
                dma_pk(6)                      # A
                dma_pk(7)                      # B
                dma_pk(8)                      # A
                dma_pk(9)                      # B
                dma_x(5, nc.sync)              # A
                for g in range(10, GC):
                    dma_pk(g)
            else:
                nc.sync.dma_start(out=bias_sb[:], in_=bias[:])
                for p in range(len(xpieces)):
                    dma_x(p, nc.gpsimd)

            # gpsimd is otherwise idle and starts earliest -> memsets there
            ones_row = persist.tile([1, 128], _BF16)
            nc.gpsimd.memset(ones_row[:], 1.0)
            warm = persist.tile([1, 512], _BF16)
            nc.gpsimd.memset(warm[:], 0.0)

            ps = [psp.tile([128, w], _F32, name=f"ps{i}")
                  for i, (_, w) in enumerate(slices)]
            ps_w = psp.tile([128, 512], _F32, name="psw")
            # clock-ramp warmup: dummy matmuls while the first DMAs land
            NWARM = 7
            for wi in range(NWARM):
                nc.tensor.matmul(ps_w[:], ones_row[:], warm[:],
                                 start=(wi == 0), stop=(wi == NWARM - 1))
            # bias seeds the accumulators (start=True)
            for si, (o0, wd_) in enumerate(slices):
                nc.tensor.matmul(ps[si][:], ones_row[:],
                                 bias_sb[:, o0:o0 + wd_],
                                 start=True, stop=False)

            if mode == "p2f8":
                for g in range(GC):
                    pk_t = pk_ts[g]
                    for qq in range(4):
                        k = g * 4 + qq
                        wd_t = wdp.tile([128, LAN], _U16)
                        nc.vector.tensor_scalar(
                            out=wd_t[:], in0=pk_t[:],
                            scalar1=2 * qq, scalar2=0xC0C0,
                            op0=mybir.AluOpType.logical_shift_left,
                            op1=mybir.AluOpType.bitwise_and)
                        last = (k == KC - 1)
                        for si, (o0, w_) in enumerate(slices):
                            rhs = wd_t[:, o0 // 2:(o0 + w_) // 2] \
                                .bitcast(_FP8)
                            nc.tensor.matmul(ps[si][:], xT_sb[:, k, :],
                                             rhs, start=False, stop=last)
            else:
                CPD = 2
                for k0 in range(0, KC, CPD):
                    w8_t = pkp.tile([128, CPD, out_core], _FP8)
                    q = nc.sync if (k0 // CPD) % 2 == 0 else nc.scalar
                    q.dma_start(
                        out=w8_t[:],
                        in_=w8[:, k0 * out_core:(k0 + CPD) * out_core]
                        .rearrange("p (c f) -> p c f", f=out_core))
                    for kk in range(CPD):
                        k = k0 + kk
                        last = (k == KC - 1)
                        for si, (o0, w_) in enumerate(slices):
                            nc.tensor.matmul(ps[si][:], xT_sb[:, k, :],
                                             w8_t[:, kk, o0:o0 + w_],
                                             start=False, stop=last)

            # split evacuation: slice 0 on DVE, slice 1 on Act, each
            # followed immediately by its own half of the output DMA
            out_sb = outp.tile([128, out_core], _BF16)
            nc.vector.tensor_copy(out=out_sb[:, 0:512], in_=ps[0][:])
            nc.sync.dma_start(out=out[:, 0:512], in_=out_sb[:, 0:512])
            nc.scalar.copy(out=out_sb[:, 512:1024], in_=ps[1][:])
            nc.scalar.dma_start(out=out[:, 512:1024],
                                in_=out_sb[:, 512:1024])

    nc.compile()
    return nc


def shard_inputs(x, weight_base, weight_tag, tile_mask, bias,
                 mode="p2f8"):
    """Host-side data prep: ternarize, shard, pack, re-layout."""
    import ml_dtypes
    in_features = x.shape[1]
    batch = x.shape[0]
    out_features = weight_base.shape[0]
    out_core = out_features // N_CORES
    KC = in_features // 128
    GC = KC // 4

    tern = _ternarize(np.asarray(weight_base, np.float32),
                      np.asarray(weight_tag, np.float32),
                      np.asarray(tile_mask, np.float32))

    xs = np.asarray(x, np.float32) * (0.5 if mode == "p2f8" else 1.0)
    xTc = np.ascontiguousarray(
        xs.T.reshape(KC, 128, batch)
        .transpose(1, 0, 2).reshape(128, KC * batch)
        .astype(ml_dtypes.bfloat16))
    bias_bf = np.asarray(bias, np.float32).astype(ml_dtypes.bfloat16)

    in_maps = []
    for c in range(N_CORES):
        o0, o1 = c * out_core, (c + 1) * out_core
        tt = tern[o0:o1, :].T                       # [in, out_core] int8
        m = {"xTc": xTc,
             "bias": np.ascontiguousarray(bias_bf[o0:o1].reshape(1, -1))}
        if mode == "p2f8":
            code = (tt & 3).astype(np.uint16)       # -1->3, 0->0, +1->1
            # code[k,j] -> chunks: k = (g*4+q)*128 + p
            code = code.reshape(GC, 4, 128, out_core)
            byte = np.zeros((GC, 128, out_core), np.uint16)
            for qq in range(4):
                byte |= code[:, qq] << np.uint16(6 - 2 * qq)
            # uint16 lane jj = cols (2jj, 2jj+1), little-endian
            pk = byte[:, :, 0::2] | (byte[:, :, 1::2] << np.uint16(8))
            m["wpk"] = np.ascontiguousarray(
                pk.transpose(1, 0, 2).reshape(128, GC * (out_core // 2)))
        else:
            w8 = tt.reshape(KC, 128, out_core).transpose(1, 0, 2) \
                .reshape(128, KC * out_core)
            m["w8"] = np.ascontiguousarray(
                w8.astype(np.float32).astype(ml_dtypes.float8_e4m3))
        in_maps.append(m)
    return in_maps, mode


_GRAPH_CACHE = {}


def _get_graph(in_features, out_core, batch, **kw):
    key = (in_features, out_core, batch, tuple(sorted(kw.items())))
    if key not in _GRAPH_CACHE:
        _GRAPH_CACHE[key] = build_graph(in_features, out_core, batch, **kw)
    return _GRAPH_CACHE[key]


def run_sharded(in_maps, trace=False, **kw):
    batch = BATCH
    in_features = in_maps[0]["xTc"].shape[1] * 128 // batch
    if "wpk" in in_maps[0]:
        out_core = in_maps[0]["wpk"].shape[1] * 8 // (in_features // 128)
    else:
        out_core = in_maps[0]["w8"].shape[1] * 128 // in_features
    nc = _get_graph(in_features, out_core, batch, **kw)
    res = run_bass_kernel_spmd(nc, in_maps, core_ids=list(range(N_CORES)),
                               trace=trace)
    full = np.concatenate([res.results[i]["out"] for i in range(N_CORES)],
                          axis=1)
    return full, res


def kernel(x, weight_base, weight_tag, tile_mask, bias):
    x = np.ascontiguousarray(np.asarray(x, dtype=np.float32))
    weight_base = np.ascontiguousarray(np.asarray(weight_base, np.float32))
    weight_tag = np.ascontiguousarray(np.asarray(weight_tag, np.float32))
    tile_mask = np.ascontiguousarray(np.asarray(tile_mask, np.float32))
    bias = np.ascontiguousarray(np.asarray(bias, np.float32))
    in_maps, mode = shard_inputs(x, weight_base, weight_tag, tile_mask,
                                 bias)
    full, _ = run_sharded(in_maps, trace=False, mode=mode)
    return np.ascontiguousarray(full.astype(np.float32))
